# revision 1
# baseline (speedup 1.0000x reference)
"""BlockMoE Trainium2 kernel (8 NeuronCores, pure data parallel).

Reference computation (per row b of x [B=65536, 1024]):
  gate:    g = relu(x @ gw1 + gb1); w = softmax(g @ gw2 + gb2)   [B, 64]
  experts: xb = x.reshape(B, 64, 16)
           h1 = relu(xb[:,e] @ ew1[e] + eb1[e])                  [B, 64, 64]
           h2 = relu(h1 @ ew2[e] + eb2[e])                       [B, 64, 32]
           l  = h2 @ ew3[e] + eb3[e]                             [B, 64]
  out:     sum_e w[:,e] * l[:,e]                                 [B, 1]

Strategy:
  - Shard batch across 8 cores (8192 rows each), replicate params.
  - Host pre-transposes x per shard into [16 tiles, 8 chunks, 128, 512]
    blocks so the kernel streams x^T directly (no PE transposes).
  - All matmuls run in float32r (full 1 col/cycle PE rate, ~1e-4 rel
    rounding vs fp32's 4 cycles/col).
  - Batch lives in the matmul free dim (N=512); contraction dims live on
    partitions, so layer outputs chain into the next layer without any
    transposes.  Per-expert weights are packed block-diagonally on host:
      L1: 32 lhsT of [128, 128] (2 experts each, K=features, M=2x64 hid)
      L2: 16 groups x 2 accumulating lhsT of [128, 128] (4 experts/group)
      L3: 16 accumulating lhsT of [128, 64] into one [64, 512] logits PSUM
  - softmax-combine folded into exp / sums:
      out = (ones^T (eg * l)) / (ones^T eg),  eg = exp(gate logits)
"""

import sys

sys.path.insert(0, "/opt/trn_rl_repo")

import numpy as np

import concourse.bass as bass
import concourse.mybir as mybir
import concourse.tile as tile
from concourse.bass_utils import run_bass_kernel_spmd

NCORES = 8
B = 65536
FULL = 1024
E = 64
WBLK = 16  # expert input block width
HID = 64
GH = 32
BL = B // NCORES  # rows per core
RT = 512  # rows per tile
NT = BL // RT  # tiles per core (16)
NCH = FULL // 128  # x^T chunks per tile (8)

F32 = mybir.dt.float32
F32R = mybir.dt.float32r
AF = mybir.ActivationFunctionType
ALU = mybir.AluOpType


def _split_multi_waits(nc, max_waits=1):
    # This walrus build rejects >1 sync-wait on one instruction; move the
    # excess onto fresh EventSemaphore instructions placed just before.
    ctr = 0
    for f in nc.m.functions:
        for blk in f.blocks:
            new_list, changed = [], False
            for inst in blk.instructions:
                si = inst.sync_info
                if si is not None and si.on_wait and len(si.on_wait) > max_waits:
                    waits = list(si.on_wait)
                    excess, keep = waits[:-max_waits], waits[-max_waits:]
                    for w in excess:
                        ev = mybir.InstEventSemaphore(
                            name=f"splitw_{ctr}", ins=[], outs=[]
                        )
                        ctr += 1
                        ev.engine = inst.engine
                        ev.sync_info = mybir.SyncInfo(on_wait=[w], on_update=[])
                        new_list.append(ev)
                    si.on_wait = keep
                    changed = True
                new_list.append(inst)
            if changed:
                blk.instructions = new_list


def _pack_params(gw1, gb1, gw2, gb2, ew1, eb1, ew2, eb2, ew3, eb3):
    """Pack parameters into the SBUF layouts the kernel DMAs verbatim."""
    # gate layer 1: lhsT chunks [128, 32] laid out as [128, 8*32]
    gw1s = np.ascontiguousarray(
        gw1.reshape(NCH, 128, GH).transpose(1, 0, 2).reshape(128, NCH * GH)
    )
    # L1: pair i = 4c + j covers experts (8c+2j, 8c+2j+1); rhs = x^T chunk c.
    # lhsT is K=128 zero-padded outside rows [32j, 32j+32).
    W1 = np.zeros((32, 128, 128), np.float32)
    for i in range(32):
        c, j = divmod(i, 4)
        e0 = 8 * c + 2 * j
        W1[i, 32 * j : 32 * j + 16, 0:64] = ew1[e0]
        W1[i, 32 * j + 16 : 32 * j + 32, 64:128] = ew1[e0 + 1]
    W1s = np.ascontiguousarray(W1.transpose(1, 0, 2).reshape(128, 32 * 128))
    # L2: chunk k = 2q + c; rhs = h1 tile (2q+c) holding experts
    # (4q+2c, 4q+2c+1); out partitions 32e..32e+32 = expert 4q+e.
    W2 = np.zeros((32, 128, 128), np.float32)
    for q in range(16):
        for c in range(2):
            k = 2 * q + c
            W2[k, 0:64, 64 * c : 64 * c + 32] = ew2[4 * q + 2 * c]
            W2[k, 64:128, 64 * c + 32 : 64 * c + 64] = ew2[4 * q + 2 * c + 1]
    W2s = np.ascontiguousarray(W2.transpose(1, 0, 2).reshape(128, 32 * 128))
    # L3: chunk q; rhs = h2 tile q (experts 4q..4q+3, 32 partitions each);
    # lhsT col (4q+e) = ew3[4q+e].
    W3 = np.zeros((16, 128, 64), np.float32)
    for q in range(16):
        for e in range(4):
            W3[q, 32 * e : 32 * e + 32, 4 * q + e] = ew3[4 * q + e][:, 0]
    W3s = np.ascontiguousarray(W3.transpose(1, 0, 2).reshape(128, 16 * 64))
    # biases as per-partition columns
    eb1s = np.ascontiguousarray(eb1.reshape(32, 128).T)  # [128, 32]
    eb2s = np.ascontiguousarray(eb2.reshape(16, 128).T)  # [128, 16]
    eb3s = np.ascontiguousarray(eb3.reshape(64, 1))  # [64, 1]
    gb1s = np.ascontiguousarray(gb1.reshape(GH, 1))  # [32, 1]
    gb2s = np.ascontiguousarray(gb2.reshape(E, 1))  # [64, 1]
    ones = np.ones((E, 1), np.float32)
    return {
        "gw1": gw1s,
        "gw2": np.ascontiguousarray(gw2),  # [32, 64]
        "w1": W1s,
        "w2": W2s,
        "w3": W3s,
        "eb1": eb1s,
        "eb2": eb2s,
        "eb3": eb3s,
        "gb1": gb1s,
        "gb2": gb2s,
        "ones": ones,
    }


def _build_nc(split=True):
    nc = bass.Bass()
    xt = nc.declare_dram_parameter("xt", [NT, NCH, 128, RT], F32R, isOutput=False)
    w1 = nc.declare_dram_parameter("w1", [128, 32 * 128], F32R, isOutput=False)
    w2 = nc.declare_dram_parameter("w2", [128, 32 * 128], F32R, isOutput=False)
    w3 = nc.declare_dram_parameter("w3", [128, 16 * 64], F32R, isOutput=False)
    gw1 = nc.declare_dram_parameter("gw1", [128, NCH * GH], F32R, isOutput=False)
    gw2 = nc.declare_dram_parameter("gw2", [GH, E], F32R, isOutput=False)
    ones = nc.declare_dram_parameter("ones", [E, 1], F32R, isOutput=False)
    eb1 = nc.declare_dram_parameter("eb1", [128, 32], F32, isOutput=False)
    eb2 = nc.declare_dram_parameter("eb2", [128, 16], F32, isOutput=False)
    eb3 = nc.declare_dram_parameter("eb3", [E, 1], F32, isOutput=False)
    gb1 = nc.declare_dram_parameter("gb1", [GH, 1], F32, isOutput=False)
    gb2 = nc.declare_dram_parameter("gb2", [E, 1], F32, isOutput=False)
    # y[t, 0:RT] = numerator, y[t, RT:2RT] = denominator; host divides.
    y = nc.declare_dram_parameter("y", [NT, 2 * RT], F32, isOutput=True)

    with tile.TileContext(nc) as tc:
        with (
            tc.tile_pool(name="consts", bufs=1) as consts,
            tc.tile_pool(name="xp", bufs=32) as xpool,
            tc.tile_pool(name="h1s", bufs=8) as h1pool,
            tc.tile_pool(name="h2s", bufs=5) as h2pool,
            tc.tile_pool(name="gsb", bufs=3) as gpool,
            tc.tile_pool(name="ph1", bufs=3, space="PSUM") as ph1,
            tc.tile_pool(name="ph2", bufs=2, space="PSUM") as ph2,
            tc.tile_pool(name="plg", bufs=1, space="PSUM") as plg,
            tc.tile_pool(name="pgate", bufs=2, space="PSUM") as pgate,
        ):
            # ---- load constants (big expert weights first: L1 of tile 0
            # needs w1 earliest after the gate phase)
            w1t = consts.tile([128, 32, 128], F32R)
            nc.sync.dma_start(w1t[:], w1[:].rearrange("p (i m) -> p i m", i=32))
            w2t = consts.tile([128, 32, 128], F32R)
            nc.sync.dma_start(w2t[:], w2[:].rearrange("p (i m) -> p i m", i=32))
            w3t = consts.tile([128, 16, 64], F32R)
            nc.sync.dma_start(w3t[:], w3[:].rearrange("p (i m) -> p i m", i=16))
            gw1t = consts.tile([128, NCH, GH], F32R)
            nc.sync.dma_start(gw1t[:], gw1[:].rearrange("p (c m) -> p c m", c=NCH))
            gw2t = consts.tile([GH, E], F32R)
            nc.sync.dma_start(gw2t[:], gw2[:])
            onest = consts.tile([E, 1], F32R)
            nc.sync.dma_start(onest[:], ones[:])
            eb1t = consts.tile([128, 32], F32)
            nc.sync.dma_start(eb1t[:], eb1[:])
            eb2t = consts.tile([128, 16], F32)
            nc.sync.dma_start(eb2t[:], eb2[:])
            eb3t = consts.tile([E, 1], F32)
            nc.sync.dma_start(eb3t[:], eb3[:])
            gb1t = consts.tile([GH, 1], F32)
            nc.sync.dma_start(gb1t[:], gb1[:])
            gb2t = consts.tile([E, 1], F32)
            nc.sync.dma_start(gb2t[:], gb2[:])

            def relu_bias(out_t, psum_t, bias_ap, use_act):
                if use_act:
                    nc.scalar.activation(out_t[:], psum_t[:], AF.Relu, bias=bias_ap)
                else:
                    nc.vector.tensor_scalar(
                        out_t[:], psum_t[:], bias_ap, 0.0, ALU.add, ALU.max
                    )

            def issue_x(t):
                tiles = []
                for c in range(NCH):
                    xc = xpool.tile([128, RT], F32R, tag="xt")
                    nc.sync.dma_start(xc[:], xt[t, c])
                    tiles.append(xc)
                return tiles

            # PE warmup: ~4us of dummy matmuls so the HAM clock gate opens
            # while the first x tiles and weights are still streaming in.
            dummy = xpool.tile([128, RT], F32R, tag="xt")
            nc.gpsimd.memzero(dummy[:])
            wp = ph1.tile([128, RT], F32, tag="h1p")
            for _ in range(80):
                nc.tensor.matmul(
                    wp[:], dummy[:, 0:128], dummy[:], start=True, stop=True
                )

            xts = issue_x(0)
            pending = None
            for t in range(NT):
                xts_next = issue_x(t + 1) if t + 1 < NT else None

                # ---- gate layer 1 (rest of the gate is emitted after the
                # expert loop so PE never stalls on ACT mid-tile)
                g1p = pgate.tile([GH, RT], F32, tag="pg")
                for c in range(NCH):
                    nc.tensor.matmul(
                        g1p[:],
                        gw1t[:, c, :],
                        xts[c][:],
                        start=(c == 0),
                        stop=(c == NCH - 1),
                    )
                g1s = gpool.tile([GH, RT], F32R, tag="g1s")
                nc.scalar.activation(g1s[:], g1p[:], AF.Relu, bias=gb1t[:, 0:1])

                # ---- experts
                lgp = plg.tile([E, RT], F32, tag="lg")
                eg = None
                for c in range(NCH):
                    if c == 1:
                        # gate layer 2 + exp, mid-stream so PE stays warm
                        g2p = pgate.tile([E, RT], F32, tag="pg")
                        nc.tensor.matmul(
                            g2p[:], gw2t[:], g1s[:], start=True, stop=True
                        )
                        eg = gpool.tile([E, RT], F32R, tag="eg")
                        nc.scalar.activation(
                            eg[:], g2p[:], AF.Exp, bias=gb2t[:, 0:1]
                        )
                    if c == 2:
                        denp = ph2.tile([1, RT], F32, tag="h2p")
                        nc.tensor.matmul(
                            denp[:], onest[:], eg[:], start=True, stop=True
                        )
                        o = gpool.tile([1, 2 * RT], F32, tag="o")
                        nc.vector.tensor_copy(o[:, RT : 2 * RT], denp[:])
                    if c == 3 and pending is not None:
                        # deferred combine tail of the previous tile: by now
                        # ls/m have long finished, so the PE never stalls
                        tp, mp, op = pending
                        nump = ph2.tile([1, RT], F32, tag="h2p")
                        nc.tensor.matmul(
                            nump[:], onest[:], mp[:], start=True, stop=True
                        )
                        nc.vector.tensor_copy(op[:, 0:RT], nump[:])
                        nc.sync.dma_start(y[tp : tp + 1, :], op[:])
                        pending = None
                    for d in range(2):  # duo of L1 pairs -> one L2 group
                        q = 2 * c + d
                        h1s_duo = []
                        for j2 in range(2):
                            j = 2 * d + j2
                            i = 4 * c + j
                            h1p = ph1.tile([128, RT], F32, tag="h1p")
                            nc.tensor.matmul(
                                h1p[:],
                                w1t[:, i, :],
                                xts[c][:],
                                start=True,
                                stop=True,
                            )
                            h1s = h1pool.tile([128, RT], F32R, tag="h1s")
                            relu_bias(h1s, h1p, eb1t[:, i : i + 1], use_act=(j == 0))
                            h1s_duo.append(h1s)
                        h2p = ph2.tile([128, RT], F32, tag="h2p")
                        nc.tensor.matmul(
                            h2p[:],
                            w2t[:, 2 * q, :],
                            h1s_duo[0][:],
                            start=True,
                            stop=False,
                        )
                        nc.tensor.matmul(
                            h2p[:],
                            w2t[:, 2 * q + 1, :],
                            h1s_duo[1][:],
                            start=False,
                            stop=True,
                        )
                        h2s = h2pool.tile([128, RT], F32R, tag="h2s")
                        relu_bias(h2s, h2p, eb2t[:, q : q + 1], use_act=True)
                        nc.tensor.matmul(
                            lgp[0:E, :],
                            w3t[:, q, :],
                            h2s[:],
                            start=(q == 0),
                            stop=(q == 15),
                        )

                # ---- combine head; the num matmul + output DMA are
                # deferred into the next tile's warm PE stream
                ls = gpool.tile([E, RT], F32R, tag="ls")
                nc.scalar.activation(ls[:], lgp[:], AF.Identity, bias=eb3t[:, 0:1])
                m = gpool.tile([E, RT], F32R, tag="m")
                nc.vector.tensor_mul(m[:], eg[:], ls[:])
                pending = (t, m, o)
                xts = xts_next
            tp, mp, op = pending
            nump = ph2.tile([1, RT], F32, tag="h2p")
            nc.tensor.matmul(nump[:], onest[:], mp[:], start=True, stop=True)
            nc.vector.tensor_copy(op[:, 0:RT], nump[:])
            nc.sync.dma_start(y[tp : tp + 1, :], op[:])

    if split:
        _split_multi_waits(nc)
    return nc


def _shard_x(x):
    """Per-core blocked transpose: [BL, 1024] -> [NT, NCH, 128, RT]."""
    shards = []
    for s in range(NCORES):
        xs = x[s * BL : (s + 1) * BL]  # [8192, 1024]
        blk = xs.reshape(NT, RT, NCH, 128).transpose(0, 2, 3, 1)
        shards.append(np.ascontiguousarray(blk))
    return shards


def run(inputs, trace=False):
    x = np.asarray(inputs["x"], np.float32)
    params = _pack_params(
        np.asarray(inputs["gw1"], np.float32),
        np.asarray(inputs["gb1"], np.float32),
        np.asarray(inputs["gw2"], np.float32),
        np.asarray(inputs["gb2"], np.float32),
        np.asarray(inputs["ew1"], np.float32),
        np.asarray(inputs["eb1"], np.float32),
        np.asarray(inputs["ew2"], np.float32),
        np.asarray(inputs["eb2"], np.float32),
        np.asarray(inputs["ew3"], np.float32),
        np.asarray(inputs["eb3"], np.float32),
    )
    xshards = _shard_x(x)
    nc = _build_nc()
    in_maps = [{"xt": xshards[s], **params} for s in range(NCORES)]
    res = run_bass_kernel_spmd(nc, in_maps, list(range(NCORES)), trace=trace)
    outs = []
    for s in range(NCORES):
        ys = res.results[s]["y"].reshape(NT, 2, RT)  # numerator, denominator
        outs.append((ys[:, 0, :] / ys[:, 1, :]).reshape(BL, 1))
    return np.concatenate(outs, axis=0), res


def kernel(**inputs) -> np.ndarray:
    out, _ = run(inputs, trace=False)
    return out



# revision 2
# speedup vs baseline: 1.0066x; 1.0066x over previous
"""BlockMoE Trainium2 kernel v2 (8 NeuronCores, data parallel).

Reference computation (per row b of x [B=65536, 1024]):
  gate:    g = relu(x @ gw1 + gb1); w = softmax(g @ gw2 + gb2)   [B, 64]
  experts: xb = x.reshape(B, 64, 16)
           h1 = relu(xb[:,e] @ ew1[e] + eb1[e])                  [B, 64, 64]
           h2 = relu(h1 @ ew2[e] + eb2[e])                       [B, 64, 32]
           l  = h2 @ ew3[e] + eb3[e]                             [B, 64]
  out:     sum_e w[:,e] * l[:,e]                                 [B, 1]

v2 strategy (vs fp32r baseline):
  - Expert L1 + L2 run in fp8e4 with MatmulPerfMode.DoubleRow: each
    instruction contracts TWO stacked K-tiles (effective K=256) at 0.5
    cycles per output column - 2x (L1, output-bound) and 4x (L2,
    K-bound) the fp32r rate.  Weights are pre-scaled x16 (powers of two
    folded through h1/h2 and unfolded exactly in bf16 ew3/256) to dodge
    fp8 subnormals.  Gate, L3 and the softmax-combine stay bf16 -
    simulated end-to-end rel_l2 ~1.1e-2 vs the 2e-2 gate.
  - L1 bias is folded INTO the DoubleRow matmul: the fp8 x^T tile has a
    9th "ones" slice, and each L1 lhsT's second K-plane carries eb1 in
    row 0.  h1 drains become pure relu on [128, 2, 512] pairs, which is
    exactly the L2 DoubleRow rhs layout.
  - PSUM->SBUF drains are spread across ACT + DVE + Pool (all three
    engines ~balanced with the PE).
  - num/den of the softmax-combine are computed as ones^T(eg*lgp) +
    eb3^T eg and ones^T eg; the host divides.
"""

import sys

sys.path.insert(0, "/opt/trn_rl_repo")

import ml_dtypes
import numpy as np

import concourse.bass as bass
import concourse.mybir as mybir
import concourse.tile as tile
from concourse.bass_utils import run_bass_kernel_spmd

NCORES = 8
B = 65536
FULL = 1024
E = 64
WBLK = 16
HID = 64
GH = 32
BL = B // NCORES  # rows per core (8192)
RT = 512  # rows per tile
NT = BL // RT  # tiles per core (16)
NCH = FULL // 128  # x^T feature chunks (8)
S = 16.0  # fp8 weight scale (power of two)

F32 = mybir.dt.float32
BF16 = mybir.dt.bfloat16
FP8 = mybir.dt.float8e4
AF = mybir.ActivationFunctionType
ALU = mybir.AluOpType
DR = mybir.MatmulPerfMode.DoubleRow

E4NP = ml_dtypes.float8_e4m3
BFNP = ml_dtypes.bfloat16

# drain engine schedules (A=ACT, D=DVE); GPSIMD cannot read PSUM on TRN2
PAIR_ENG = "ADADADADADADADAA"  # h1 pair-drains, q=0..15 (9A/7D)
H2_ENG = "DADADADADADADADA"  # h2 drains, q=0..15 (8A/8D)


def _split_multi_waits(nc, max_waits=1):
    # This walrus build rejects >1 sync-wait on one instruction; move the
    # excess onto fresh EventSemaphore instructions placed just before.
    ctr = 0
    for f in nc.m.functions:
        for blk in f.blocks:
            new_list, changed = [], False
            for inst in blk.instructions:
                si = inst.sync_info
                if si is not None and si.on_wait and len(si.on_wait) > max_waits:
                    waits = list(si.on_wait)
                    excess, keep = waits[:-max_waits], waits[-max_waits:]
                    for w in excess:
                        ev = mybir.InstEventSemaphore(
                            name=f"splitw_{ctr}", ins=[], outs=[]
                        )
                        ctr += 1
                        ev.engine = inst.engine
                        ev.sync_info = mybir.SyncInfo(on_wait=[w], on_update=[])
                        new_list.append(ev)
                    si.on_wait = keep
                    changed = True
                new_list.append(inst)
            if changed:
                blk.instructions = new_list


def _pack_params(gw1, gb1, gw2, gb2, ew1, eb1, ew2, eb2, ew3, eb3):
    """Pack parameters into the SBUF layouts the kernel DMAs verbatim."""
    f8 = lambda a: np.ascontiguousarray(a.astype(np.float32)).astype(E4NP)
    bf = lambda a: np.ascontiguousarray(a.astype(np.float32)).astype(BFNP)

    # L1 DoubleRow lhsT: [128, 4 pairs, 8 instrs, 2 kplanes, 128 cols].
    # Instr (p,u) covers experts e0=16p+2u, e0+1; kplane 0 multiplies x^T
    # chunk (2p + (u>=4)), kplane 1 multiplies the ones slice (bias row 0).
    W1 = np.zeros((128, 4, 8, 2, 128), np.float32)
    for p in range(4):
        for u in range(8):
            e0 = 16 * p + 2 * u
            r = 32 * (u % 4)
            W1[r : r + 16, p, u, 0, 0:64] = S * ew1[e0]
            W1[r + 16 : r + 32, p, u, 0, 64:128] = S * ew1[e0 + 1]
            W1[r, p, u, 1, 0:64] = S * eb1[e0]
            W1[r, p, u, 1, 64:128] = S * eb1[e0 + 1]
    # L2 DoubleRow lhsT: [128, 16 instrs, 2 kplanes, 128 cols]. Instr q
    # contracts h1 pair q (kplane j = h1 tile 2q+j, experts 4q+2j(+1) on
    # partitions 0:64 / 64:128); out cols 32e..32e+32 = expert 4q+e.
    W2 = np.zeros((128, 16, 2, 128), np.float32)
    for q in range(16):
        for j in range(2):
            W2[0:64, q, j, 64 * j : 64 * j + 32] = S * ew2[4 * q + 2 * j]
            W2[64:128, q, j, 64 * j + 32 : 64 * j + 64] = S * ew2[4 * q + 2 * j + 1]
    # L3 bf16 lhsT: [128, 16, 64]; rhs = h2s tile q (4 experts x 32 hid at
    # scale S^2); col 4q+e = ew3[4q+e] / S^2.
    W3 = np.zeros((128, 16, 64), np.float32)
    for q in range(16):
        for e in range(4):
            W3[32 * e : 32 * e + 32, q, 4 * q + e] = ew3[4 * q + e][:, 0] / (S * S)
    # h2 drain biases: column q partitions 32e..32e+32 = S^2 * eb2[4q+e]
    EB2 = np.zeros((128, 16), np.float32)
    for q in range(16):
        EB2[:, q] = (S * S) * eb2[4 * q : 4 * q + 4].reshape(128)
    gw1c = gw1.reshape(NCH, 128, GH).transpose(1, 0, 2)  # [128, 8, 32]
    return {
        "w1": f8(W1),
        "w2": f8(W2),
        "w3": bf(W3),
        "gw1": bf(gw1c),
        "gw2": bf(gw2),
        "eb2": np.ascontiguousarray(EB2),
        "gb1": np.ascontiguousarray(gb1.reshape(GH, 1).astype(np.float32)),
        "gb2": np.ascontiguousarray(gb2.reshape(E, 1).astype(np.float32)),
        "onesb": np.concatenate(
            [np.ones((E, 1), np.float32), eb3.reshape(E, 1).astype(np.float32)],
            axis=1,
        ).astype(BFNP),
    }


def _build_nc(nt=NT, split=True):
    nc = bass.Bass()
    xb = nc.declare_dram_parameter("xb", [nt, 128, NCH, RT], BF16, isOutput=False)
    x8 = nc.declare_dram_parameter("x8", [nt, 128, NCH + 1, RT], FP8, isOutput=False)
    w1 = nc.declare_dram_parameter("w1", [128, 4, 8, 2, 128], FP8, isOutput=False)
    w2 = nc.declare_dram_parameter("w2", [128, 16, 2, 128], FP8, isOutput=False)
    w3 = nc.declare_dram_parameter("w3", [128, 16, 64], BF16, isOutput=False)
    gw1 = nc.declare_dram_parameter("gw1", [128, NCH, GH], BF16, isOutput=False)
    gw2 = nc.declare_dram_parameter("gw2", [GH, E], BF16, isOutput=False)
    eb2 = nc.declare_dram_parameter("eb2", [128, 16], F32, isOutput=False)
    gb1 = nc.declare_dram_parameter("gb1", [GH, 1], F32, isOutput=False)
    gb2 = nc.declare_dram_parameter("gb2", [E, 1], F32, isOutput=False)
    onesb = nc.declare_dram_parameter("onesb", [E, 2], BF16, isOutput=False)
    # y[t, 0] = den, y[t, 1] = eb3^T eg, y[t, 2] = ones^T m;
    # host computes (y1 + y2) / y0.
    y = nc.declare_dram_parameter("y", [nt, 3, RT], F32, isOutput=True)

    with tile.TileContext(nc) as tc:
        with (
            tc.tile_pool(name="consts", bufs=1) as consts,
            tc.tile_pool(name="xbp", bufs=2) as xbpool,
            tc.tile_pool(name="x8p", bufs=2) as x8pool,
            tc.tile_pool(name="h1s", bufs=5) as h1pool,
            tc.tile_pool(name="h2s", bufs=5) as h2pool,
            tc.tile_pool(name="gsb", bufs=2) as gpool,
            tc.tile_pool(name="ph1", bufs=2, space="PSUM") as ph1,
            tc.tile_pool(name="ph2", bufs=2, space="PSUM") as ph2,
            tc.tile_pool(name="pg", bufs=1, space="PSUM") as pgate,
            tc.tile_pool(name="plg", bufs=1, space="PSUM") as plg,
        ):
            # ---- constants (order: earliest-needed first)
            gw1t = consts.tile([128, NCH, GH], BF16)
            nc.sync.dma_start(gw1t[:], gw1[:])
            xb0t = xbpool.tile([128, NCH, RT], BF16, tag="xb")
            nc.sync.dma_start(xb0t[:], xb[0])
            x80t = x8pool.tile([128, NCH + 1, RT], FP8, tag="x8")
            nc.sync.dma_start(x80t[:], x8[0])
            w1t = consts.tile([128, 4, 8, 2, 128], FP8)
            nc.sync.dma_start(w1t[:], w1[:])
            w2t = consts.tile([128, 16, 2, 128], FP8)
            nc.sync.dma_start(w2t[:], w2[:])
            w3t = consts.tile([128, 16, 64], BF16)
            nc.sync.dma_start(w3t[:], w3[:])
            gw2t = consts.tile([GH, E], BF16)
            nc.sync.dma_start(gw2t[:], gw2[:])
            eb2t = consts.tile([128, 16], F32)
            nc.sync.dma_start(eb2t[:], eb2[:])
            gb1t = consts.tile([GH, 1], F32)
            nc.sync.dma_start(gb1t[:], gb1[:])
            gb2t = consts.tile([E, 1], F32)
            nc.sync.dma_start(gb2t[:], gb2[:])
            onest = consts.tile([E, 2], BF16)
            nc.sync.dma_start(onest[:], onesb[:])

            engs = {
                "A": nc.scalar,
                "D": nc.vector,
                "P": nc.gpsimd,
            }

            def drain_relu(eng, out_t, psum_ap, bias_ap=None):
                if bias_ap is None:
                    if eng == "A":
                        nc.scalar.activation(out_t, psum_ap, AF.Relu)
                    else:
                        engs[eng].tensor_scalar_max(out_t, psum_ap, 0.0)
                else:
                    if eng == "A":
                        nc.scalar.activation(out_t, psum_ap, AF.Relu, bias=bias_ap)
                    else:
                        engs[eng].tensor_scalar(
                            out_t, psum_ap, bias_ap, 0.0, ALU.add, ALU.max
                        )

            def issue_x(t):
                xbt = xbpool.tile([128, NCH, RT], BF16, tag="xb")
                nc.sync.dma_start(xbt[:], xb[t])
                x8t = x8pool.tile([128, NCH + 1, RT], FP8, tag="x8")
                nc.sync.dma_start(x8t[:], x8[t])
                return xbt, x8t

            # PE warmup: dummy matmuls so the HW clock ramps while the first
            # x tiles and weights stream in.
            dummy = gpool.tile([128, RT], BF16, tag="dum")
            nc.gpsimd.memzero(dummy[:])
            wp = ph1.tile([128, 2, RT], F32, tag="h1p")
            for _ in range(14):
                nc.tensor.matmul(
                    wp[:, 0, :], dummy[:, 0:128], dummy[:], start=True, stop=True
                )

            xts = (xb0t, x80t)
            pending = None
            g1s_next = None
            for t in range(nt):
                xbt, x8t = xts
                xts_next = issue_x(t + 1) if t + 1 < nt else None

                # gate1 for tile 0 runs at the top of its own body; for
                # t>0 it was emitted before the previous tile's tail (fills
                # the drain-wait gap there)
                g1s = g1s_next

                # ---- expert pipeline over q = 0..15
                lgp = plg.tile([128, RT], F32, tag="lg")
                eg = None
                h1tiles = [None] * 16
                h2tiles = [None] * 16

                def l1_pair(q):
                    h1p = ph1.tile([128, 2, RT], F32, tag="h1p")
                    for j in range(2):
                        i = 2 * q + j
                        p, u = i // 8, i % 8
                        c = 2 * p + (1 if u >= 4 else 0)
                        r = 32 * (u % 4)
                        nc.tensor.matmul(
                            h1p[:, j, :],
                            w1t[r : r + 32, p, u],
                            x8t[r : r + 32, c : NCH + 1 : NCH - c, :],
                            start=True,
                            stop=True,
                            perf_mode=DR,
                            tile_position=(r, 0),
                        )
                    h1s = h1pool.tile([128, 2, RT], FP8, tag="h1s")
                    drain_relu(PAIR_ENG[q], h1s[:], h1p[:])
                    h1tiles[q] = h1s

                def l2_mm(q):
                    h2p = ph2.tile([128, RT], F32, tag="h2p")
                    nc.tensor.matmul(
                        h2p[:],
                        w2t[:, q],
                        h1tiles[q][:],
                        start=True,
                        stop=True,
                        perf_mode=DR,
                    )
                    h2s = h2pool.tile([128, RT], BF16, tag="h2s")
                    drain_relu(H2_ENG[q], h2s[:], h2p[:], eb2t[:, q : q + 1])
                    h2tiles[q] = h2s

                def l3_mm(q):
                    nc.tensor.matmul(
                        lgp[0:E, :],
                        w3t[:, q, :],
                        h2tiles[q][:],
                        start=(q == 0),
                        stop=(q == 15),
                    )

                def gate1(xbt_n):
                    g1p = pgate.tile([GH, RT], F32, tag="pg")
                    for c in range(NCH):
                        nc.tensor.matmul(
                            g1p[:],
                            gw1t[:, c, :],
                            xbt_n[:, c, :],
                            start=(c == 0),
                            stop=(c == NCH - 1),
                        )
                    g1s = gpool.tile([GH, RT], BF16, tag="g1s")
                    nc.scalar.activation(g1s[:], g1p[:], AF.Relu, bias=gb1t[:, 0:1])
                    return g1s

                if g1s is None:
                    g1s = gate1(xbt)

                for q in range(16):
                    if q == 2:
                        eg = gpool.tile([E, RT], BF16, tag="eg")
                        nc.scalar.activation(eg[:], g2p[:], AF.Exp, bias=gb2t[:, 0:1])
                    l1_pair(q)
                    if q == 1:
                        g2p = pgate.tile([E, RT], F32, tag="pg")
                        nc.tensor.matmul(
                            g2p[:], gw2t[:], g1s[:], start=True, stop=True
                        )
                    if q == 3:
                        # [den; eb3^T eg] -> lgp partitions 64,65
                        nc.tensor.matmul(
                            lgp[64:66, :], onest[:], eg[:], start=True, stop=True
                        )
                        od = gpool.tile([2, RT], F32, tag="od")
                        nc.vector.tensor_copy(od[:], lgp[64:66, :])
                        nc.sync.dma_start(y[t, 0:2], od[:])
                    if q == 4 and pending is not None:
                        # deferred combine tail of the previous tile
                        tp, mp = pending
                        nump = pgate.tile([1, RT], F32, tag="pg")
                        nc.tensor.matmul(
                            nump[:], onest[:, 0:1], mp[:], start=True, stop=True
                        )
                        on = gpool.tile([1, RT], F32, tag="on")
                        nc.vector.tensor_copy(on[:], nump[:])
                        nc.sync.dma_start(y[tp, 2:3], on[:])
                        pending = None
                    if q >= 2:
                        l2_mm(q - 2)
                    if q >= 4:
                        l3_mm(q - 4)
                # tail: next tile's gate1 is emitted first so the PE chews
                # on it while the last drains land; then L2(14..15), L3(12..15)
                g1s_next = gate1(xts_next[0]) if xts_next is not None else None
                l2_mm(14)
                l3_mm(12)
                l2_mm(15)
                l3_mm(13)
                l3_mm(14)
                l3_mm(15)

                # m = eg * lgp (deferred num matmul reads it next tile)
                m = gpool.tile([E, RT], BF16, tag="m")
                nc.vector.tensor_tensor(m[:], eg[:], lgp[0:E, :], ALU.mult)
                pending = (t, m)
                xts = xts_next

            tp, mp = pending
            nump = pgate.tile([1, RT], F32, tag="pg")
            nc.tensor.matmul(nump[:], onest[:, 0:1], mp[:], start=True, stop=True)
            on = gpool.tile([1, RT], F32, tag="on")
            nc.vector.tensor_copy(on[:], nump[:])
            nc.sync.dma_start(y[tp, 2:3], on[:])

    if split:
        _split_multi_waits(nc)
    return nc


def _shard_x(x, nt=NT):
    """Per-core x tiles: bf16 [nt,128,8,512] and fp8(+ones) [nt,128,9,512]."""
    rows = nt * RT
    outs = []
    for s in range(NCORES):
        xs = x[s * BL : s * BL + rows]  # [rows, 1024]
        blk = xs.reshape(nt, RT, NCH, 128).transpose(0, 3, 2, 1)  # [nt,128,8,512]
        xbf = np.ascontiguousarray(blk).astype(BFNP)
        x8 = np.empty((nt, 128, NCH + 1, RT), E4NP)
        x8[:, :, :NCH, :] = blk.astype(E4NP)
        x8[:, :, NCH, :] = np.float32(1.0)
        outs.append((xbf, x8))
    return outs


def run(inputs, trace=False):
    x = np.asarray(inputs["x"], np.float32)
    params = _pack_params(
        *[
            np.asarray(inputs[k], np.float32)
            for k in [
                "gw1", "gb1", "gw2", "gb2",
                "ew1", "eb1", "ew2", "eb2", "ew3", "eb3",
            ]
        ]
    )
    xshards = _shard_x(x)
    nc = _build_nc()
    in_maps = [
        {"xb": xshards[s][0], "x8": xshards[s][1], **params} for s in range(NCORES)
    ]
    res = run_bass_kernel_spmd(nc, in_maps, list(range(NCORES)), trace=trace)
    outs = []
    for s in range(NCORES):
        ys = np.asarray(res.results[s]["y"])  # [NT, 3, RT]: den, eb3eg, onesm
        outs.append(((ys[:, 1, :] + ys[:, 2, :]) / ys[:, 0, :]).reshape(BL, 1))
    return np.concatenate(outs, axis=0), res


def kernel(**inputs) -> np.ndarray:
    out, _ = run(inputs, trace=False)
    return out


# revision 3
# speedup vs baseline: 1.0128x; 1.0062x over previous
"""BlockMoE Trainium2 kernel v2 (8 NeuronCores, data parallel).

Reference computation (per row b of x [B=65536, 1024]):
  gate:    g = relu(x @ gw1 + gb1); w = softmax(g @ gw2 + gb2)   [B, 64]
  experts: xb = x.reshape(B, 64, 16)
           h1 = relu(xb[:,e] @ ew1[e] + eb1[e])                  [B, 64, 64]
           h2 = relu(h1 @ ew2[e] + eb2[e])                       [B, 64, 32]
           l  = h2 @ ew3[e] + eb3[e]                             [B, 64]
  out:     sum_e w[:,e] * l[:,e]                                 [B, 1]

v2 strategy (vs fp32r baseline):
  - Expert L1 + L2 run in fp8e4 with MatmulPerfMode.DoubleRow: each
    instruction contracts TWO stacked K-tiles (effective K=256) at 0.5
    cycles per output column - 2x (L1, output-bound) and 4x (L2,
    K-bound) the fp32r rate.  Weights are pre-scaled x16 (powers of two
    folded through h1/h2 and unfolded exactly in bf16 ew3/256) to dodge
    fp8 subnormals.  Gate, L3 and the softmax-combine stay bf16 -
    simulated end-to-end rel_l2 ~1.1e-2 vs the 2e-2 gate.
  - L1 bias is folded INTO the DoubleRow matmul: the fp8 x^T tile has a
    9th "ones" slice, and each L1 lhsT's second K-plane carries eb1 in
    row 0.  h1 drains become pure relu on [128, 2, 512] pairs, which is
    exactly the L2 DoubleRow rhs layout.
  - PSUM->SBUF drains are spread across ACT + DVE + Pool (all three
    engines ~balanced with the PE).
  - num/den of the softmax-combine are computed as ones^T(eg*lgp) +
    eb3^T eg and ones^T eg; the host divides.
"""

import sys

sys.path.insert(0, "/opt/trn_rl_repo")

import ml_dtypes
import numpy as np

import concourse.bass as bass
import concourse.mybir as mybir
import concourse.tile as tile
from concourse.bass_utils import run_bass_kernel_spmd

NCORES = 8
B = 65536
FULL = 1024
E = 64
WBLK = 16
HID = 64
GH = 32
BL = B // NCORES  # rows per core (8192)
RT = 512  # rows per tile
NT = BL // RT  # tiles per core (16)
NCH = FULL // 128  # x^T feature chunks (8)
S = 16.0  # fp8 weight scale (power of two)

F32 = mybir.dt.float32
BF16 = mybir.dt.bfloat16
FP8 = mybir.dt.float8e4
AF = mybir.ActivationFunctionType
ALU = mybir.AluOpType
DR = mybir.MatmulPerfMode.DoubleRow

E4NP = ml_dtypes.float8_e4m3
BFNP = ml_dtypes.bfloat16

# drain engine schedules (A=ACT, D=DVE); GPSIMD cannot read PSUM on TRN2
PAIR_ENG = "ADADADADADADADAA"  # h1 pair-drains, q=0..15 (9A/7D)
H2_ENG = "DADADADADADADADA"  # h2 drains, q=0..15 (8A/8D)


def _split_multi_waits(nc, max_waits=1):
    # This walrus build rejects >1 sync-wait on one instruction; move the
    # excess onto fresh EventSemaphore instructions placed just before.
    ctr = 0
    for f in nc.m.functions:
        for blk in f.blocks:
            new_list, changed = [], False
            for inst in blk.instructions:
                si = inst.sync_info
                if si is not None and si.on_wait and len(si.on_wait) > max_waits:
                    waits = list(si.on_wait)
                    excess, keep = waits[:-max_waits], waits[-max_waits:]
                    for w in excess:
                        ev = mybir.InstEventSemaphore(
                            name=f"splitw_{ctr}", ins=[], outs=[]
                        )
                        ctr += 1
                        ev.engine = inst.engine
                        ev.sync_info = mybir.SyncInfo(on_wait=[w], on_update=[])
                        new_list.append(ev)
                    si.on_wait = keep
                    changed = True
                new_list.append(inst)
            if changed:
                blk.instructions = new_list


def _pack_params(gw1, gb1, gw2, gb2, ew1, eb1, ew2, eb2, ew3, eb3):
    """Pack parameters into the SBUF layouts the kernel DMAs verbatim."""
    f8 = lambda a: np.ascontiguousarray(a.astype(np.float32)).astype(E4NP)
    bf = lambda a: np.ascontiguousarray(a.astype(np.float32)).astype(BFNP)

    # L1 DoubleRow lhsT: [128, 4 pairs, 8 instrs, 2 kplanes, 128 cols].
    # Instr (p,u) covers experts e0=16p+2u, e0+1; kplane 0 multiplies x^T
    # chunk (2p + (u>=4)), kplane 1 multiplies the ones slice (bias row 0).
    W1 = np.zeros((128, 4, 8, 2, 128), np.float32)
    for p in range(4):
        for u in range(8):
            e0 = 16 * p + 2 * u
            r = 32 * (u % 4)
            W1[r : r + 16, p, u, 0, 0:64] = S * ew1[e0]
            W1[r + 16 : r + 32, p, u, 0, 64:128] = S * ew1[e0 + 1]
            W1[r, p, u, 1, 0:64] = S * eb1[e0]
            W1[r, p, u, 1, 64:128] = S * eb1[e0 + 1]
    # L2 DoubleRow lhsT: [128, 16 instrs, 2 kplanes, 128 cols]. Instr q
    # contracts h1 pair q (kplane j = h1 tile 2q+j, experts 4q+2j(+1) on
    # partitions 0:64 / 64:128); out cols 32e..32e+32 = expert 4q+e.
    W2 = np.zeros((128, 16, 2, 128), np.float32)
    for q in range(16):
        for j in range(2):
            W2[0:64, q, j, 64 * j : 64 * j + 32] = S * ew2[4 * q + 2 * j]
            W2[64:128, q, j, 64 * j + 32 : 64 * j + 64] = S * ew2[4 * q + 2 * j + 1]
    # L3 bf16 lhsT: [128, 16, 64]; rhs = h2s tile q (4 experts x 32 hid at
    # scale S^2); col 4q+e = ew3[4q+e] / S^2.
    W3 = np.zeros((128, 16, 64), np.float32)
    for q in range(16):
        for e in range(4):
            W3[32 * e : 32 * e + 32, q, 4 * q + e] = ew3[4 * q + e][:, 0] / (S * S)
    # h2 drain biases: column q partitions 32e..32e+32 = S^2 * eb2[4q+e]
    EB2 = np.zeros((128, 16), np.float32)
    for q in range(16):
        EB2[:, q] = (S * S) * eb2[4 * q : 4 * q + 4].reshape(128)
    gw1c = gw1.reshape(NCH, 128, GH).transpose(1, 0, 2)  # [128, 8, 32]
    # merge constants into one DMA per dtype (each dma_start pays ~1us of
    # SWDGE issue time on the SP queue; 11 DMAs serialized the startup)
    cb8 = np.concatenate(
        [W1.reshape(128, 8192), W2.reshape(128, 4096)], axis=1
    )
    gw2p = np.zeros((128, E), np.float32)
    gw2p[0:GH] = gw2
    oep = np.zeros((128, 2), np.float32)
    oep[0:E, 0] = 1.0
    oep[0:E, 1] = eb3[:, 0]
    cbf = np.concatenate(
        [W3.reshape(128, 1024), gw1c.reshape(128, 256), gw2p, oep], axis=1
    )
    cf32 = np.zeros((128, 18), np.float32)
    cf32[:, 0:16] = EB2
    cf32[0:GH, 16] = gb1
    cf32[0:E, 17] = gb2
    return {
        "cb8": f8(cb8),
        "cbf": bf(cbf),
        "cf32": np.ascontiguousarray(cf32),
    }


def _build_nc(nt=NT, split=True):
    nc = bass.Bass()
    xb = nc.declare_dram_parameter("xb", [nt, 128, NCH, RT], BF16, isOutput=False)
    x8 = nc.declare_dram_parameter("x8", [nt, 128, NCH + 1, RT], FP8, isOutput=False)
    cb8 = nc.declare_dram_parameter("cb8", [128, 12288], FP8, isOutput=False)
    cbf = nc.declare_dram_parameter("cbf", [128, 1346], BF16, isOutput=False)
    cf32 = nc.declare_dram_parameter("cf32", [128, 18], F32, isOutput=False)
    # y[t, 0] = den, y[t, 1] = eb3^T eg, y[t, 2] = ones^T m;
    # host computes (y1 + y2) / y0.
    y = nc.declare_dram_parameter("y", [nt, 3, RT], F32, isOutput=True)

    with tile.TileContext(nc) as tc:
        with (
            tc.tile_pool(name="consts", bufs=1) as consts,
            tc.tile_pool(name="xbp", bufs=2) as xbpool,
            tc.tile_pool(name="x8p", bufs=2) as x8pool,
            tc.tile_pool(name="h1s", bufs=5) as h1pool,
            tc.tile_pool(name="h2s", bufs=5) as h2pool,
            tc.tile_pool(name="gsb", bufs=2) as gpool,
            tc.tile_pool(name="ph1", bufs=2, space="PSUM") as ph1,
            tc.tile_pool(name="ph2", bufs=2, space="PSUM") as ph2,
            tc.tile_pool(name="pg", bufs=1, space="PSUM") as pgate,
            tc.tile_pool(name="plg", bufs=1, space="PSUM") as plg,
        ):
            # ---- constants: one DMA per dtype, typed slice views
            cbft = consts.tile([128, 1346], BF16)
            nc.sync.dma_start(cbft[:], cbf[:])
            xb0t = xbpool.tile([128, NCH, RT], BF16, tag="xb")
            nc.sync.dma_start(xb0t[:], xb[0])
            x80t = x8pool.tile([128, NCH + 1, RT], FP8, tag="x8")
            nc.sync.dma_start(x80t[:], x8[0])
            cb8t = consts.tile([128, 12288], FP8)
            nc.sync.dma_start(cb8t[:], cb8[:])
            cf32t = consts.tile([128, 18], F32)
            nc.sync.dma_start(cf32t[:], cf32[:])
            w1t = cb8t[:, 0:8192].rearrange(
                "p (a u j m) -> p a u j m", a=4, u=8, j=2
            )
            w2t = cb8t[:, 8192:12288].rearrange(
                "p (q j m) -> p q j m", q=16, j=2
            )
            w3t = cbft[:, 0:1024].rearrange("p (q m) -> p q m", q=16)
            gw1t = cbft[:, 1024:1280].rearrange("p (c m) -> p c m", c=NCH)
            gw2t = cbft[0:GH, 1280:1344]
            onest = cbft[0:E, 1344:1346]
            eb2t = cf32t[:, 0:16]
            gb1t = cf32t[0:GH, 16:17]
            gb2t = cf32t[0:E, 17:18]

            engs = {
                "A": nc.scalar,
                "D": nc.vector,
                "P": nc.gpsimd,
            }

            def drain_relu(eng, out_t, psum_ap, bias_ap=None):
                if bias_ap is None:
                    if eng == "A":
                        nc.scalar.activation(out_t, psum_ap, AF.Relu)
                    else:
                        engs[eng].tensor_scalar_max(out_t, psum_ap, 0.0)
                else:
                    if eng == "A":
                        nc.scalar.activation(out_t, psum_ap, AF.Relu, bias=bias_ap)
                    else:
                        engs[eng].tensor_scalar(
                            out_t, psum_ap, bias_ap, 0.0, ALU.add, ALU.max
                        )

            def issue_x(t):
                xbt = xbpool.tile([128, NCH, RT], BF16, tag="xb")
                nc.sync.dma_start(xbt[:], xb[t])
                x8t = x8pool.tile([128, NCH + 1, RT], FP8, tag="x8")
                nc.sync.dma_start(x8t[:], x8[t])
                return xbt, x8t

            # PE warmup: dummy matmuls so the HW clock ramps while the first
            # x tiles and weights stream in.
            dummy = gpool.tile([128, RT], BF16, tag="dum")
            nc.gpsimd.memzero(dummy[:])
            wp = ph1.tile([128, 2, RT], F32, tag="h1p")
            for _ in range(14):
                nc.tensor.matmul(
                    wp[:, 0, :], dummy[:, 0:128], dummy[:], start=True, stop=True
                )

            xts = (xb0t, x80t)
            pending = None
            g1s_next = None
            for t in range(nt):
                xbt, x8t = xts
                xts_next = issue_x(t + 1) if t + 1 < nt else None

                # gate1 for tile 0 runs at the top of its own body; for
                # t>0 it was emitted before the previous tile's tail (fills
                # the drain-wait gap there)
                g1s = g1s_next

                # ---- expert pipeline over q = 0..15
                lgp = plg.tile([128, RT], F32, tag="lg")
                eg = None
                h1tiles = [None] * 16
                h2tiles = [None] * 16

                def l1_pair(q):
                    h1p = ph1.tile([128, 2, RT], F32, tag="h1p")
                    for j in range(2):
                        i = 2 * q + j
                        p, u = i // 8, i % 8
                        c = 2 * p + (1 if u >= 4 else 0)
                        r = 32 * (u % 4)
                        nc.tensor.matmul(
                            h1p[:, j, :],
                            w1t[r : r + 32, p, u],
                            x8t[r : r + 32, c : NCH + 1 : NCH - c, :],
                            start=True,
                            stop=True,
                            perf_mode=DR,
                            tile_position=(r, 0),
                        )
                    h1s = h1pool.tile([128, 2, RT], FP8, tag="h1s")
                    drain_relu(PAIR_ENG[q], h1s[:], h1p[:])
                    h1tiles[q] = h1s

                def l2_mm(q):
                    h2p = ph2.tile([128, RT], F32, tag="h2p")
                    nc.tensor.matmul(
                        h2p[:],
                        w2t[:, q],
                        h1tiles[q][:],
                        start=True,
                        stop=True,
                        perf_mode=DR,
                    )
                    h2s = h2pool.tile([128, RT], BF16, tag="h2s")
                    drain_relu(H2_ENG[q], h2s[:], h2p[:], eb2t[:, q : q + 1])
                    h2tiles[q] = h2s

                def l3_mm(q):
                    nc.tensor.matmul(
                        lgp[0:E, :],
                        w3t[:, q, :],
                        h2tiles[q][:],
                        start=(q == 0),
                        stop=(q == 15),
                    )

                def gate1(xbt_n):
                    g1p = pgate.tile([GH, RT], F32, tag="pg")
                    for c in range(NCH):
                        nc.tensor.matmul(
                            g1p[:],
                            gw1t[:, c, :],
                            xbt_n[:, c, :],
                            start=(c == 0),
                            stop=(c == NCH - 1),
                        )
                    g1s = gpool.tile([GH, RT], BF16, tag="g1s")
                    nc.scalar.activation(g1s[:], g1p[:], AF.Relu, bias=gb1t[:, 0:1])
                    return g1s

                if g1s is None:
                    g1s = gate1(xbt)

                for q in range(16):
                    if q == 2:
                        eg = gpool.tile([E, RT], BF16, tag="eg")
                        nc.scalar.activation(eg[:], g2p[:], AF.Exp, bias=gb2t[:, 0:1])
                    l1_pair(q)
                    if q == 1:
                        g2p = pgate.tile([E, RT], F32, tag="pg")
                        nc.tensor.matmul(
                            g2p[:], gw2t[:], g1s[:], start=True, stop=True
                        )
                    if q == 3:
                        # [den; eb3^T eg] -> lgp partitions 64,65
                        nc.tensor.matmul(
                            lgp[64:66, :], onest[:], eg[:], start=True, stop=True
                        )
                        od = gpool.tile([2, RT], F32, tag="od")
                        nc.vector.tensor_copy(od[:], lgp[64:66, :])
                        nc.sync.dma_start(y[t, 0:2], od[:])
                    if q == 4 and pending is not None:
                        # deferred combine tail of the previous tile
                        tp, mp = pending
                        nump = pgate.tile([1, RT], F32, tag="pg")
                        nc.tensor.matmul(
                            nump[:], onest[:, 0:1], mp[:], start=True, stop=True
                        )
                        on = gpool.tile([1, RT], F32, tag="on")
                        nc.vector.tensor_copy(on[:], nump[:])
                        nc.sync.dma_start(y[tp, 2:3], on[:])
                        pending = None
                    if q >= 2:
                        l2_mm(q - 2)
                    if q >= 4:
                        l3_mm(q - 4)
                # tail: next tile's gate1 is emitted first so the PE chews
                # on it while the last drains land; then L2(14..15), L3(12..15)
                g1s_next = gate1(xts_next[0]) if xts_next is not None else None
                l2_mm(14)
                l3_mm(12)
                l2_mm(15)
                l3_mm(13)
                l3_mm(14)
                l3_mm(15)

                # m = eg * lgp (deferred num matmul reads it next tile)
                m = gpool.tile([E, RT], BF16, tag="m")
                nc.vector.tensor_tensor(m[:], eg[:], lgp[0:E, :], ALU.mult)
                pending = (t, m)
                xts = xts_next

            tp, mp = pending
            nump = pgate.tile([1, RT], F32, tag="pg")
            nc.tensor.matmul(nump[:], onest[:, 0:1], mp[:], start=True, stop=True)
            on = gpool.tile([1, RT], F32, tag="on")
            nc.vector.tensor_copy(on[:], nump[:])
            nc.sync.dma_start(y[tp, 2:3], on[:])

    if split:
        _split_multi_waits(nc)
    return nc


def _shard_x(x, nt=NT):
    """Per-core x tiles: bf16 [nt,128,8,512] and fp8(+ones) [nt,128,9,512]."""
    rows = nt * RT
    outs = []
    for s in range(NCORES):
        xs = x[s * BL : s * BL + rows]  # [rows, 1024]
        blk = xs.reshape(nt, RT, NCH, 128).transpose(0, 3, 2, 1)  # [nt,128,8,512]
        xbf = np.ascontiguousarray(blk).astype(BFNP)
        x8 = np.empty((nt, 128, NCH + 1, RT), E4NP)
        x8[:, :, :NCH, :] = blk.astype(E4NP)
        x8[:, :, NCH, :] = np.float32(1.0)
        outs.append((xbf, x8))
    return outs


def run(inputs, trace=False):
    x = np.asarray(inputs["x"], np.float32)
    params = _pack_params(
        *[
            np.asarray(inputs[k], np.float32)
            for k in [
                "gw1", "gb1", "gw2", "gb2",
                "ew1", "eb1", "ew2", "eb2", "ew3", "eb3",
            ]
        ]
    )
    xshards = _shard_x(x)
    nc = _build_nc()
    in_maps = [
        {"xb": xshards[s][0], "x8": xshards[s][1], **params} for s in range(NCORES)
    ]
    res = run_bass_kernel_spmd(nc, in_maps, list(range(NCORES)), trace=trace)
    outs = []
    for s in range(NCORES):
        ys = np.asarray(res.results[s]["y"])  # [NT, 3, RT]: den, eb3eg, onesm
        outs.append(((ys[:, 1, :] + ys[:, 2, :]) / ys[:, 0, :]).reshape(BL, 1))
    return np.concatenate(outs, axis=0), res


def kernel(**inputs) -> np.ndarray:
    out, _ = run(inputs, trace=False)
    return out
